# revision 11
# baseline (speedup 1.0000x reference)
"""Multi-head causal attention (Whisper-style) on 8 trn2 NeuronCores.

Sharding: head-parallel (2 of 16 heads per core) + row-parallel output
projection.  Each core receives the full (host-transposed) activations
x^T, its column slice of Wq/Wk/Wv (+bias slices) and its row slice of
Wo, and produces a full-size partial y^T = (o_heads @ Wo_rows)^T.  The
host sums the 8 partials, transposes back, and adds bo (the standard
post-allreduce bias placement for row-parallel layers).

On-chip layout is fully "transposed" (feature dim on partitions):
  q^T,k^T = Wq/k^T-free matmuls over x^T  -> [128, S] per batch
  scores^T[k,q] per (batch, head), softmax along partitions via a
  ones-column appended to v (denominator rides the o^T matmul), no
  max-subtraction (exp(-1e9 + s) underflows to exactly 0).
Mask blocks are classified host-side by value: all < -50 -> block
skipped (contributes exactly 0 after exp), all == 0 -> exp directly,
else -> the mask tile is DMA'd and added before exp.
"""

import os
import sys
from contextlib import ExitStack

import numpy as np

for _p in ("/root/.axon_site/_ro/trn_rl_repo", "/opt/trn_rl_repo"):
    if os.path.isdir(_p) and _p not in sys.path:
        sys.path.append(_p)

import concourse.bass as bass
import concourse.mybir as mybir
import concourse.tile as tile
from concourse import bacc, bass_utils

F32 = mybir.dt.float32
F32R = mybir.dt.float32r
AF = mybir.ActivationFunctionType
ALU = mybir.AluOpType

N_STATE = 1024
N_HEAD = 16
HD = 64
N_CORES = 8
HEADS_PER_CORE = N_HEAD // N_CORES  # 2
E = HEADS_PER_CORE * HD  # 128 feature columns per core
Q_TILE = 512
K_CHUNK = 128
N_D = N_STATE // 128  # 8 contraction chunks for the projections
SCALE = float(HD) ** -0.25
NEG_THRESH = -50.0


def classify_blocks(maskT):
    """Value-driven classification of (k_chunk, q_tile) mask blocks."""
    S = maskT.shape[0]
    cls = {}
    for ki in range(S // K_CHUNK):
        for j in range(S // Q_TILE):
            blk = maskT[ki * K_CHUNK:(ki + 1) * K_CHUNK,
                        j * Q_TILE:(j + 1) * Q_TILE]
            if np.all(blk < NEG_THRESH):
                cls[(ki, j)] = "skip"
            elif np.all(blk == 0.0):
                cls[(ki, j)] = "clean"
            else:
                cls[(ki, j)] = "partial"
    return cls


def build_kernel(B, S, cls):
    """Build the per-core SPMD Bass program (identical on all cores)."""
    n_k = S // K_CHUNK
    n_q = S // Q_TILE
    n_t = S // Q_TILE  # token chunks per batch for the projections

    nc = bacc.Bacc("TRN2", target_bir_lowering=False, debug=False,
                   num_devices=N_CORES)

    xT_d = nc.dram_tensor("xT", [B, N_STATE, S], F32R, kind="ExternalInput")
    maskT_d = nc.dram_tensor("maskT", [S, S], F32, kind="ExternalInput")
    wq_d = nc.dram_tensor("wq", [N_STATE, E], F32R, kind="ExternalInput")
    wk_d = nc.dram_tensor("wk", [N_STATE, E], F32R, kind="ExternalInput")
    wv_d = nc.dram_tensor("wv", [N_STATE, E], F32R, kind="ExternalInput")
    wo_d = nc.dram_tensor("wo", [E, N_STATE], F32R, kind="ExternalInput")
    bq_d = nc.dram_tensor("bq", [E], F32, kind="ExternalInput")
    ident_d = nc.dram_tensor("ident", [128, 128], F32R, kind="ExternalInput")
    bv_d = nc.dram_tensor("bv", [E], F32, kind="ExternalInput")
    yT_d = nc.dram_tensor("yT", [B, N_STATE, S], F32, kind="ExternalOutput")

    partial_blocks = sorted(k for k, v in cls.items() if v == "partial")

    with tile.TileContext(nc) as tc, ExitStack() as ctx:
        const = ctx.enter_context(tc.tile_pool(name="const", bufs=1))
        xpool = ctx.enter_context(tc.tile_pool(name="xpool", bufs=2))
        stage = ctx.enter_context(tc.tile_pool(name="stage", bufs=3))
        wexp = ctx.enter_context(tc.tile_pool(name="wexp", bufs=4))
        bcast = ctx.enter_context(tc.tile_pool(name="bcast", bufs=2))
        psA = ctx.enter_context(tc.tile_pool(name="psA", bufs=2, space="PSUM"))
        psS = ctx.enter_context(tc.tile_pool(name="psS", bufs=2, space="PSUM"))
        psO = ctx.enter_context(tc.tile_pool(name="psO", bufs=4, space="PSUM"))

        # ---- resident constants / weights ----
        ident = const.tile([128, 128], F32R, tag="ident")
        nc.sync.dma_start(ident[:], ident_d[:])
        wq_sb = const.tile([128, N_D, E], F32R, tag="wq_sb")
        wk_sb = const.tile([128, N_D, E], F32R, tag="wk_sb")
        wv_sb = const.tile([128, N_D, E], F32R, tag="wv_sb")
        for w_sb, w_d in ((wq_sb, wq_d), (wk_sb, wk_d), (wv_sb, wv_d)):
            nc.sync.dma_start(
                w_sb[:], w_d[:].rearrange("(c p) e -> p c e", p=128))
        wo_sb = const.tile([E, N_STATE], F32R, tag="wo_sb")
        nc.sync.dma_start(wo_sb[:], wo_d[:])
        bq_sb = const.tile([E, 1], F32, tag="bq_sb")
        bv_sb = const.tile([E, 1], F32, tag="bv_sb")
        nc.sync.dma_start(bq_sb[:], bq_d[:].rearrange("(e o) -> e o", o=1))
        nc.sync.dma_start(bv_sb[:], bv_d[:].rearrange("(e o) -> e o", o=1))

        # resident mask tiles for partial blocks
        mask_sb = {}
        for (ki, j) in partial_blocks:
            mt = const.tile([K_CHUNK, Q_TILE], F32, name=f"mask_{ki}_{j}",
                            tag=f"mask_{ki}_{j}")
            nc.sync.dma_start(
                mt[:], maskT_d[ki * K_CHUNK:(ki + 1) * K_CHUNK,
                               j * Q_TILE:(j + 1) * Q_TILE])
            mask_sb[(ki, j)] = mt

        # resident activations
        qT = [const.tile([E, S], F32R, name=f"qT{b}", tag=f"qT{b}")
              for b in range(B)]
        kT = [const.tile([E, S], F32R, name=f"kT{b}", tag=f"kT{b}")
              for b in range(B)]
        onT = [const.tile([E, S], F32R, name=f"onT{b}", tag=f"onT{b}")
               for b in range(B)]
        vn = [const.tile([128, n_k, 2 * (HD + 1)], F32R, name=f"vn{b}",
                         tag=f"vn{b}") for b in range(B)]

        # ---- stage A: projections ----
        for b in range(B):
            nc.vector.memset(vn[b][:].bitcast(F32), 1.0)  # ones cols for denoms
            for j in range(n_t):
                ts = slice(j * Q_TILE, (j + 1) * Q_TILE)
                xt = xpool.tile([128, N_D, Q_TILE], F32R, tag="xt")
                nc.sync.dma_start(
                    xt[:],
                    xT_d[b, :, ts].rearrange("(c p) t -> p c t", p=128))
                for proj, w_sb in (("q", wq_sb), ("k", wk_sb), ("v", wv_sb)):
                    ps = psA.tile([E, Q_TILE], F32, tag="psA")
                    for c in range(N_D):
                        nc.tensor.matmul(ps[:], w_sb[:, c, :], xt[:, c, :],
                                         start=(c == 0), stop=(c == N_D - 1))
                    if proj == "q":
                        nc.vector.tensor_scalar(
                            qT[b][:, ts], ps[:], bq_sb[:], SCALE,
                            ALU.add, ALU.mult)
                    elif proj == "k":
                        nc.scalar.activation(kT[b][:, ts], ps[:], AF.Copy,
                                             scale=SCALE)
                    else:
                        vs = stage.tile([E, Q_TILE], F32R, tag="vstage")
                        nc.vector.tensor_scalar(
                            vs[:], ps[:], bv_sb[:], None, ALU.add)
                        for c in range(Q_TILE // 128):
                            tp = psS.tile([128, 128], F32R, tag="psS")
                            nc.tensor.matmul(tp[:], vs[:, c * 128:(c + 1) * 128],
                                             ident[:], is_transpose=True)
                            ci = j * (Q_TILE // 128) + c
                            nc.vector.tensor_copy(vn[b][:, ci, 0:HD],
                                                  tp[:, 0:HD])
                            nc.vector.tensor_copy(vn[b][:, ci, HD + 1:2 * HD + 1],
                                                  tp[:, HD:2 * HD])

        # ---- stage B: attention per (batch, head) ----
        for b in range(B):
            for h in range(HEADS_PER_CORE):
                hs = slice(h * HD, (h + 1) * HD)
                vslice = slice(h * (HD + 1), (h + 1) * (HD + 1))
                ot = [psO.tile([HD + 1, Q_TILE], F32, tag="psO",
                               name=f"ot{b}_{h}_{j}") for j in range(n_q)]
                first = [True] * n_q
                for ki in range(n_k):
                    ks = slice(ki * K_CHUNK, (ki + 1) * K_CHUNK)
                    for j in range(n_q):
                        kind = cls[(ki, j)]
                        if kind == "skip":
                            continue
                        qs = slice(j * Q_TILE, (j + 1) * Q_TILE)
                        sc = psS.tile([K_CHUNK, Q_TILE], F32, tag="psS")
                        nc.tensor.matmul(sc[:], kT[b][hs, ks], qT[b][hs, qs],
                                         start=True, stop=True)
                        wt = wexp.tile([K_CHUNK, Q_TILE], F32R, tag="wexp")
                        if kind == "partial":
                            sm = stage.tile([K_CHUNK, Q_TILE], F32, tag="smask")
                            nc.vector.tensor_add(sm[:], sc[:],
                                                 mask_sb[(ki, j)][:])
                            nc.scalar.activation(wt[:], sm[:], AF.Exp)
                        else:
                            nc.scalar.activation(wt[:], sc[:], AF.Exp)
                        last = not any(
                            cls[(ki2, j)] != "skip" for ki2 in range(ki + 1, n_k))
                        nc.tensor.matmul(ot[j][:], vn[b][:, ki, vslice], wt[:],
                                         start=first[j], stop=last)
                        first[j] = False
                for j in range(n_q):
                    qs = slice(j * Q_TILE, (j + 1) * Q_TILE)
                    rd = stage.tile([1, Q_TILE], F32, tag="rd")
                    nc.vector.reciprocal(rd[:], ot[j][HD:HD + 1, :])
                    bc = bcast.tile([HD, Q_TILE], F32, tag="bc")
                    rdap = rd[:]
                    rd_rep = bass.AP(rdap.tensor, rdap.offset,
                                     [list(rdap.ap[0]), [0, HD],
                                      list(rdap.ap[1])])
                    nc.sync.dma_start(bc[:], rd_rep)
                    nc.vector.tensor_tensor(onT[b][hs, qs], ot[j][0:HD, :],
                                            bc[:], ALU.mult)

        # ---- stage C: output projection (row-parallel partial) ----
        for b in range(B):
            for m in range(N_STATE // 128):
                ms = slice(m * 128, (m + 1) * 128)
                for j in range(n_q):
                    qs = slice(j * Q_TILE, (j + 1) * Q_TILE)
                    yp = psA.tile([128, Q_TILE], F32, tag="psA")
                    nc.tensor.matmul(yp[:], wo_sb[:, ms], onT[b][:, qs],
                                     start=True, stop=True)
                    ys = stage.tile([128, Q_TILE], F32, tag="ystage")
                    nc.any.tensor_copy(ys[:], yp[:])
                    nc.sync.dma_start(yT_d[b, ms, qs], ys[:])

    nc.finalize()
    return nc


def shard_inputs(x, mask, Wq, bq, Wk, Wv, bv, Wo):
    """Per-core input dicts (host-side layout prep + slicing only)."""
    B = x.shape[0]
    xT = np.ascontiguousarray(x.transpose(0, 2, 1)).astype(np.float32)
    maskT = np.ascontiguousarray(mask.T).astype(np.float32)
    in_maps = []
    for c in range(N_CORES):
        cs = slice(c * E, (c + 1) * E)
        in_maps.append({
            "xT": xT,
            "maskT": maskT,
            "wq": np.ascontiguousarray(Wq[:, cs]),
            "wk": np.ascontiguousarray(Wk[:, cs]),
            "wv": np.ascontiguousarray(Wv[:, cs]),
            "wo": np.ascontiguousarray(Wo[cs, :]),
            "bq": np.ascontiguousarray(bq[cs]),
            "bv": np.ascontiguousarray(bv[cs]),
            "ident": np.eye(128, dtype=np.float32),
        })
    return in_maps


_NC_CACHE = {}


def _get_nc(B, S, cls_key, cls):
    key = (B, S, cls_key)
    if key not in _NC_CACHE:
        _NC_CACHE[key] = build_kernel(B, S, cls)
    return _NC_CACHE[key]


def run(x, mask, Wq, bq, Wk, Wv, bv, Wo, bo, trace=False):
    B, S, D = x.shape
    maskT = np.ascontiguousarray(np.asarray(mask).T).astype(np.float32)
    cls = classify_blocks(maskT)
    cls_key = tuple(sorted((k, v) for k, v in cls.items()))
    nc = _get_nc(B, S, hash(cls_key), cls)
    in_maps = shard_inputs(np.asarray(x, np.float32), maskT.T,
                           np.asarray(Wq, np.float32), np.asarray(bq, np.float32),
                           np.asarray(Wk, np.float32), np.asarray(Wv, np.float32),
                           np.asarray(bv, np.float32), np.asarray(Wo, np.float32))
    res = bass_utils.run_bass_kernel_spmd(
        nc, in_maps, core_ids=list(range(N_CORES)), trace=trace)
    acc = np.zeros((B, N_STATE, S), dtype=np.float64)
    for r in res.results:
        acc += r["yT"].astype(np.float64)
    y = acc.transpose(0, 2, 1).astype(np.float32) + np.asarray(bo, np.float32)
    return y, res


def kernel(x, mask, Wq, bq, Wk, Wv, bv, Wo, bo):
    y, _ = run(x, mask, Wq, bq, Wk, Wv, bv, Wo, bo, trace=False)
    return y


def time_run(x, mask, Wq, bq, Wk, Wv, bv, Wo, bo, iters=20):
    """Measure per-iteration device execution time of the SPMD program.

    Mirrors bass2jax.run_bass_via_pjrt's multi-core lowering, but keeps
    inputs device-resident and chains donated output buffers so `iters`
    executions pipeline back-to-back; returns (y, seconds_per_iter).
    """
    import time as _time
    import jax
    from jax.experimental.shard_map import shard_map
    from jax.sharding import Mesh, NamedSharding, PartitionSpec
    from concourse import bass2jax
    from concourse.bass2jax import _bass_exec_p, install_neuronx_cc_hook

    install_neuronx_cc_hook()
    B, S, D = x.shape
    maskT = np.ascontiguousarray(np.asarray(mask).T).astype(np.float32)
    cls = classify_blocks(maskT)
    cls_key = tuple(sorted((k, v) for k, v in cls.items()))
    nc = _get_nc(B, S, hash(cls_key), cls)
    in_maps = shard_inputs(np.asarray(x, np.float32), maskT.T,
                           np.asarray(Wq, np.float32), np.asarray(bq, np.float32),
                           np.asarray(Wk, np.float32), np.asarray(Wv, np.float32),
                           np.asarray(bv, np.float32), np.asarray(Wo, np.float32))

    in_names, out_names, out_avals, zero_outs = [], [], [], []
    partition_name = (nc.partition_id_tensor.name
                      if nc.partition_id_tensor else None)
    for alloc in nc.m.functions[0].allocations:
        if not isinstance(alloc, mybir.MemoryLocationSet):
            continue
        name = alloc.memorylocations[0].name
        if alloc.kind == "ExternalInput":
            if name != partition_name:
                in_names.append(name)
        elif alloc.kind == "ExternalOutput":
            out_names.append(name)
            shape = tuple(alloc.tensor_shape)
            dtype = mybir.dt.np(alloc.dtype)
            out_avals.append((shape, dtype))
            zero_outs.append(np.zeros(shape, dtype))
    n_params = len(in_names)
    n_outs = len(out_names)
    all_in_names = list(in_names) + list(out_names)
    if partition_name is not None:
        all_in_names.append(partition_name)

    def _body(*args):
        operands = list(args)
        if partition_name is not None:
            operands.append(bass2jax.partition_id_tensor())
        outs = _bass_exec_p.bind(
            *operands,
            out_avals=tuple(
                jax.core.ShapedArray(s, d) for s, d in out_avals),
            in_names=tuple(all_in_names),
            out_names=tuple(out_names),
            lowering_input_output_aliases=(),
            sim_require_finite=True,
            sim_require_nnan=True,
            nc=nc,
        )
        return tuple(outs)

    devices = jax.devices()[:N_CORES]
    mesh = Mesh(np.asarray(devices), ("core",))
    spec = PartitionSpec("core")
    donate = tuple(range(n_params, n_params + n_outs))
    sharded = jax.jit(
        shard_map(_body, mesh=mesh, in_specs=(spec,) * (n_params + n_outs),
                  out_specs=(spec,) * n_outs, check_rep=False),
        donate_argnums=donate, keep_unused=True)

    sh = NamedSharding(mesh, spec)
    dev_in = [
        jax.device_put(
            np.concatenate([np.asarray(in_maps[c][nm]) for c in range(N_CORES)],
                           axis=0), sh)
        for nm in in_names
    ]
    out = sharded(*dev_in, *[
        jax.device_put(np.zeros((N_CORES * z.shape[0], *z.shape[1:]), z.dtype),
                       sh) for z in zero_outs])
    jax.block_until_ready(out)  # warmup + compile
    t0 = _time.perf_counter()
    for _ in range(iters):
        out = sharded(*dev_in, *out)
    jax.block_until_ready(out)
    dt = (_time.perf_counter() - t0) / iters

    yT_all = np.asarray(out[out_names.index("yT")])
    acc = np.zeros((B, N_STATE, S), dtype=np.float64)
    for c in range(N_CORES):
        acc += yT_all.reshape(N_CORES, B, N_STATE, S)[c].astype(np.float64)
    y = acc.transpose(0, 2, 1).astype(np.float32) + np.asarray(bo, np.float32)
    return y, dt
